# revision 1
# baseline (speedup 1.0000x reference)
"""Multi-head self-attention with RoPE — Trainium2 Bass kernel, 8 NeuronCores.

Sharding: core c = 2*b + g handles batch b = c//2 and head-group g = c%2
(8 of the 16 heads).  Within each batch pair the cores AllGather their
normalized attention outputs (O^T, bf16) and both run the full output
projection redundantly; the host keeps the even core's y.  No reduction
on the host.

Two kernel bodies exist: _emit (version=1, the active default) and
_emit2 (version=2, experimental).  version=2 restructures the schedule
(lq-outer software pipelining with piece injection, own-half projection
+ pairwise ReduceScatter of y, PSUM->SBUF copies on DVE, PE warmup); it
passes the multi-core interpreter at rel_err 7e-3 but produces 100%
NaN on real hardware.  Ruled out by bisection (each fixed/reverted and
still NaN): gpsimd reading PSUM (BIR verifier rejects; moved to DVE),
collective writing the IO tensor (rejected; staged via Internal DRAM),
DVE stream_shuffle RoPE (reverted to swap DMAs), reciprocal from PSUM
(copy-first), in-place reciprocal (separate tile), sliced/rearranged
input DMA APs (monolithic DMAs), collectives themselves (stand-in DMA
build still NaNs).  Remaining suspects: the PE warmup junk matmuls,
the per-head oT2 gather DMAs, or a scheduling-exposed race the
interpreter's conservative ordering hides.  Keep version=1 until
isolated.

Per-core dataflow (matmuls bf16, fp32 PSUM accumulation):
  xT [E, L] bf16 (pre-transposed on host)
  QKV:   Q^T/K^T pair tiles via W-stationary matmuls; V natural [L, 512].
  RoPE:  weights pre-permuted on host to de-interleave even/odd dims, so
         rotate-half becomes a 32-partition block swap (SBUF->SBUF DMA);
         cos/sin multiplies on GPSIMD, combine add on DVE.
  Scores:S^T half-tiles [Lk=128, Lq=512] per head, heads row-tiled on PE;
         three halves share a [128, 1536] PSUM tile (double buffered) so
         exp runs as few, wide ACT instructions overlapped with PE.
  Softmax: denominator via ones-column appended to V (PSUM partition 64
         of O^T); reciprocal_approx + gpsimd partition_broadcast.
  AV:    O^T[65, 512] += V_aug^T A^T over 16 Lk chunks.
  Proj:  y[lq] = Ocat^T.T @ w_out, fp32 [L, E].
"""

import contextlib
import functools

import numpy as np
import ml_dtypes

import concourse.bass as bass
import concourse.mybir as mybir
import concourse.tile as tile
from concourse import bacc
from concourse.bass_utils import run_bass_kernel_spmd

BF16 = mybir.dt.bfloat16
F32 = mybir.dt.float32
N_CORES = 8
ROPE_THETA = 10000.0

B_FULL, L_FULL, E_FULL = 4, 2048, 1024
H_FULL = 16


def _emit(tc, nc, xT, wqkv, wout, cosT, sinT, y, L, E, HC, D, taps=None, use_collective=True, rep=0):
    P = 128
    EC = E // P                 # E chunks of 128 (contraction)
    NPAIR = HC // 2             # head pairs per core
    LT = L // 512               # 512-wide L tiles
    LKC = L // P                # 128-wide Lk chunks
    A = HC * D                  # local attention width (512)
    scale = 1.0 / float(np.sqrt(D))
    Exp = mybir.ActivationFunctionType.Exp

    ctx = contextlib.ExitStack()
    pool = ctx.enter_context(tc.tile_pool(name="sb", bufs=1))
    psum = ctx.enter_context(tc.tile_pool(name="ps", bufs=1, space="PSUM"))
    work = ctx.enter_context(tc.tile_pool(name="wk", bufs=1))
    dram = ctx.enter_context(tc.tile_pool(name="dr", bufs=1, space="DRAM"))

    # ---- persistent SBUF buffers ----
    xt_sb = pool.tile([P, EC, L], BF16, tag="xbuf")
    wqkv_sb = pool.tile([P, EC, 3 * A], BF16, tag="wqkv")
    wout_sb = pool.tile([P, EC, E], BF16, tag="wout")
    cos_sb = pool.tile([P, L], BF16, tag="costab")
    sin_sb = pool.tile([P, L], BF16, tag="sintab")
    qk_sb = pool.tile([P, 2, NPAIR, L], BF16, tag="qk")      # [pair-rows, q/k, pair, L]
    vaug_sb = pool.tile([P, LKC, HC, D + 1], BF16, tag="vaug")
    ot_sb = pool.tile([64, HC, L], BF16, tag="ot")           # normalized O^T per head

    nc.sync.dma_start(wqkv_sb[:], wqkv.ap().rearrange("(c p) n -> p c n", p=P))
    for xc in range(LT):  # x in L-chunks so the V matmuls start early
        nc.sync.dma_start(
            xt_sb[:, :, xc * 512 : (xc + 1) * 512],
            xT.ap()[:, xc * 512 : (xc + 1) * 512].rearrange("(c p) l -> p c l", p=P),
        )
    nc.sync.dma_start(cos_sb[:], cosT.ap())
    nc.sync.dma_start(sin_sb[:], sinT.ap())
    nc.sync.dma_start(wout_sb[:], wout.ap().rearrange("(c p) n -> p c n", p=P))

    # ones column for the softmax denominator
    nc.vector.memset(vaug_sb[:, :, :, D : D + 1], 1.0)

    # ---- V = x @ Wv, natural [L, A] layout, 2 L-chunks per PSUM tile ----
    for vg in range(LKC // 2):
        ps = psum.tile([P, 1024], F32, tag="sc", bufs=2)
        for i in range(2):
            lt = vg * 2 + i
            for e in range(EC):
                nc.tensor.matmul(
                    ps[:, i * 512 : (i + 1) * 512],
                    lhsT=xt_sb[:, e, lt * P : (lt + 1) * P],
                    rhs=wqkv_sb[:, e, 2 * A : 3 * A],
                    start=(e == 0),
                    stop=(e == EC - 1),
                )
        nc.scalar.copy(
            out=vaug_sb[:, vg * 2 : (vg + 1) * 2, :, 0:D],
            in_=ps[:].rearrange("p (t h d) -> p t h d", h=HC, d=D),
        )

    # ---- Q^T / K^T + RoPE ----
    # psum tile cols: [q | k] for one 512-wide L tile
    for p in range(NPAIR):
        for lt in range(LT):
            ps = psum.tile([P, 1024], F32, tag="sc", bufs=2)
            for qk in range(2):
                wcol = qk * A + p * P
                for e in range(EC):
                    nc.tensor.matmul(
                        ps[:, qk * 512 : (qk + 1) * 512],
                        lhsT=wqkv_sb[:, e, wcol : wcol + P],
                        rhs=xt_sb[:, e, lt * 512 : (lt + 1) * 512],
                        start=(e == 0),
                        stop=(e == EC - 1),
                    )
            Lsl = slice(lt * 512, (lt + 1) * 512)
            tab = lambda sb: (
                sb[:, Lsl][:, None, :].to_broadcast([P, 2, 512])
            )
            qs = work.tile([P, 1024], BF16, tag="qs", bufs=3)
            nc.scalar.copy(out=qs[:], in_=ps[:])
            qs_v = qs[:].rearrange("p (q c) -> p q c", q=2)
            w = work.tile([P, 1024], BF16, tag="w", bufs=3)
            t = work.tile([P, 1024], BF16, tag="w", bufs=3)
            nc.gpsimd.tensor_mul(w[:].rearrange("p (q c) -> p q c", q=2), qs_v, tab(sin_sb))
            nc.gpsimd.tensor_mul(t[:].rearrange("p (q c) -> p q c", q=2), qs_v, tab(cos_sb))
            wsw = work.tile([P, 1024], BF16, tag="wsw", bufs=2)
            for blk in range(4):
                sb = blk ^ 1  # swap 32-row blocks pairwise
                nc.sync.dma_start(
                    wsw[blk * 32 : (blk + 1) * 32, :], w[sb * 32 : (sb + 1) * 32, :]
                )
            out_ap = qk_sb[:, :, p, Lsl]  # [P, 2, 512]
            nc.vector.tensor_add(
                out_ap,
                t[:].rearrange("p (q c) -> p q c", q=2),
                wsw[:].rearrange("p (q c) -> p q c", q=2),
            )

    # ---- attention + AllGather staging ----
    cc_half = NPAIR // 2 * P  # feature rows per collective (2 pairs x 128)
    cc_in = [
        dram.tile([cc_half, L], BF16, tag=f"ccin{i}", bufs=1, name=f"ccin{i}_{rep}")
        for i in range(2)
    ]
    cc_out = [
        dram.tile([2, cc_half, L], BF16, tag=f"ccout{i}", bufs=1, name=f"ccout{i}_{rep}")
        for i in range(2)
    ]

    for p in range(NPAIR):
        h0, h1 = 2 * p, 2 * p + 1
        for lq in range(LT):
            ot0 = psum.tile([65, 512], F32, tag="ot", bufs=2)
            ot1 = psum.tile([65, 512], F32, tag="ot", bufs=2)
            ots = (ot0, ot1)
            Lq = slice(lq * 512, (lq + 1) * 512)
            # halves: (head, lk) pairs in lk-major order, grouped 3 per
            # [128, 1536] psum tile so exp runs as wide ACT instructions.
            halves = [(hh, lk) for lk in range(LKC) for hh in range(2)]
            gi = 0
            while gi < len(halves):
                grp = halves[gi : gi + 3]
                nh = len(grp)
                ps = psum.tile([P, 1536], F32, tag="sc", bufs=2)
                for j, (hh, lk) in enumerate(grp):
                    nc.tensor.matmul(
                        ps[:, j * 512 : (j + 1) * 512],
                        lhsT=qk_sb[hh * 64 : (hh + 1) * 64, 1, p, lk * P : (lk + 1) * P],
                        rhs=qk_sb[hh * 64 : (hh + 1) * 64, 0, p, Lq],
                        start=True,
                        stop=True,
                    )
                at = work.tile([P, 1536], BF16, tag="at", bufs=4)
                nc.scalar.activation(at[:, : nh * 512], ps[:, : nh * 512], Exp, scale=scale)
                for j, (hh, lk) in enumerate(grp):
                    nc.tensor.matmul(
                        ots[hh][:],
                        lhsT=vaug_sb[:, lk, 2 * p + hh, :],
                        rhs=at[:, j * 512 : (j + 1) * 512],
                        start=(lk == 0),
                        stop=(lk == LKC - 1),
                    )
                gi += nh
            for hh, otp in ((0, ot0), (1, ot1)):
                # denominator: PSUM row 64 -> SBUF row 64 -> (DMA) row 0 ->
                # reciprocal -> broadcast to 64 partitions -> scale O^T.
                den = work.tile([65, 512], F32, tag="den", bufs=1)
                nc.vector.tensor_copy(out=den[64:65, :], in_=otp[64:65, :])
                den0 = work.tile([1, 512], F32, tag="den0", bufs=1)
                nc.sync.dma_start(den0[0:1, :], den[64:65, :])
                rec0 = work.tile([1, 512], F32, tag="rec0", bufs=1)
                nc.vector.reciprocal_approx_fast(rec0[0:1, :], den0[0:1, :])
                rbc = work.tile([64, 512], F32, tag="rbc", bufs=2)
                nc.gpsimd.partition_broadcast(rbc[:], rec0[0:1, :])
                nc.vector.tensor_mul(ot_sb[:, 2 * p + hh, Lq], otp[0:64, :], rbc[:])
        if p % 2 == 1:
            half = p // 2
            src = ot_sb[:, half * (NPAIR // 2) * 2 : (half + 1) * (NPAIR // 2) * 2, :]
            nc.sync.dma_start(
                cc_in[half][:].rearrange("(h d) l -> d h l", d=64),
                src,
            )
            if use_collective:
                nc.gpsimd.collective_compute(
                    "AllGather",
                    mybir.AluOpType.bypass,
                    replica_groups=[[2 * i, 2 * i + 1] for i in range(N_CORES // 2)],
                    ins=[cc_in[half][:].opt()],
                    outs=[cc_out[half][:].opt()],
                )
            else:  # timing-analysis build: stand-in DMAs, no collective
                nc.sync.dma_start(cc_out[half][0], cc_in[half][:])
                nc.sync.dma_start(cc_out[half][1], cc_in[half][:])

    # ---- gather Ocat^T into SBUF: [128, EC, L], global feature-major ----
    ocat_sb = pool.tile([P, EC, L], BF16, tag="xbuf")
    for half in range(2):
        for g2 in range(2):
            blk = cc_out[half][g2]  # [cc_half, L]
            for q in range(cc_half // P):
                f = g2 * (2 * cc_half) + half * cc_half + q * P  # global row
                nc.sync.dma_start(ocat_sb[:, f // P, :], blk[q * P : (q + 1) * P, :])

    if taps is not None:
        nc.sync.dma_start(taps["ot"].ap(), ot_sb[:])
        nc.sync.dma_start(
            taps["ocat"].ap().rearrange("(c p) l -> p c l", p=P), ocat_sb[:]
        )
        nc.sync.dma_start(taps["qk"].ap(), qk_sb[:])
        nc.sync.dma_start(taps["vaug"].ap(), vaug_sb[:])

    # ---- output projection: y[lq] = Ocat^T.T @ wout (full L, redundant) ----
    for lq in range(L // P):
        ps = psum.tile([P, 1024], F32, tag="sc", bufs=2)
        for nhf in range(E // 512):
            for e in range(EC):
                nc.tensor.matmul(
                    ps[:, nhf * 512 : (nhf + 1) * 512],
                    lhsT=ocat_sb[:, e, lq * P : (lq + 1) * P],
                    rhs=wout_sb[:, e, nhf * 512 : (nhf + 1) * 512],
                    start=(e == 0),
                    stop=(e == EC - 1),
                )
        yt = work.tile([P, E], F32, tag="yt", bufs=2)
        nc.scalar.copy(out=yt[:], in_=ps[:, :E])
        nc.sync.dma_start(y.ap()[lq * P : (lq + 1) * P, :], yt[:])

    ctx.close()


def _emit2(tc, nc, xT, wqkv, wout, cosT, sinT, y, L, E, HC, D,
           use_collective=True, rep=0, pack_sim=False):
    """v2: flat software-pipelined emission.

    - lq-outer attention (pairs inner) so the per-lq output projection
      overlaps the next tile's ACT-bound softmax.
    - own-half output projection + pairwise ReduceScatter(add) of y
      (w_out row-sliced per core; y per core is [L/2, E]).
    - exp groups of 2 halves ([128,1024] PSUM, 4 banks for sc x2) so the
      O^T accumulators can quad-buffer (4 banks) -- avoids pair-boundary
      stalls on the softmax-denominator chain.
    - scores staggered one group ahead of AV so PE never queues behind
      the exp it feeds; QKV+RoPE of pair p+1 / proj of tile lq-1 are
      injected between attention groups to fill PE under ACT.
    - PSUM->SBUF copies on DVE (ACT does only exp).
    """
    P = 128
    EC = E // P
    NPAIR = HC // 2
    LT = L // 512
    LKC = L // P
    A = HC * D
    scale = 1.0 / float(np.sqrt(D))
    Exp = mybir.ActivationFunctionType.Exp

    ctx = contextlib.ExitStack()
    pool = ctx.enter_context(tc.tile_pool(name="sb", bufs=1))
    psum = ctx.enter_context(tc.tile_pool(name="ps", bufs=1, space="PSUM"))
    work = ctx.enter_context(tc.tile_pool(name="wk", bufs=1))
    dram = ctx.enter_context(tc.tile_pool(name="dr", bufs=1, space="DRAM"))

    xt_sb = pool.tile([P, EC, L], BF16, tag="xbuf")
    wqkv_sb = pool.tile([P, EC, 3 * A], BF16, tag="wqkv")
    wout_sb = pool.tile([P, A // P, E], BF16, tag="wout")  # own rows only
    cos_sb = pool.tile([P, L], BF16, tag="costab")
    sin_sb = pool.tile([P, L], BF16, tag="sintab")
    qk_sb = pool.tile([P, 2, NPAIR, L], BF16, tag="qk")
    vaug_sb = pool.tile([P, LKC, HC, D + 1], BF16, tag="vaug")
    ot_sb = pool.tile([64, HC, L], BF16, tag="ot")

    # bisect build: monolithic input DMAs (v1-style)
    nc.sync.dma_start(xt_sb[:], xT.ap().rearrange("(c p) l -> p c l", p=P))
    nc.sync.dma_start(wqkv_sb[:], wqkv.ap().rearrange("(c p) n -> p c n", p=P))
    nc.sync.dma_start(cos_sb[:], cosT.ap())
    nc.sync.dma_start(sin_sb[:], sinT.ap())
    nc.sync.dma_start(wout_sb[:], wout.ap().rearrange("(c p) n -> p c n", p=P))

    nc.vector.memset(vaug_sb[:, :, :, D : D + 1], 1.0)

    def v_half(lt):
        """V rows for one 128-row L chunk: 8 accumulating MMs + DVE copy."""
        ps = psum.tile([P, 1024], F32, tag="sc", bufs=2)
        for e in range(EC):
            nc.tensor.matmul(
                ps[:, :512],
                lhsT=xt_sb[:, e, lt * P : (lt + 1) * P],
                rhs=wqkv_sb[:, e, 2 * A : 3 * A],
                start=(e == 0),
                stop=(e == EC - 1),
            )
        nc.vector.tensor_copy(
            out=vaug_sb[:, lt, :, 0:D],
            in_=ps[:, :512].rearrange("p (h d) -> p h d", h=HC),
        )

    def qk_half(p, lt, half):
        """Q (half=0) or K (half=1) of pair p for one 512-wide L tile,
        with the RoPE chain entirely on DVE + one swap DMA."""
        ps = psum.tile([P, 1024], F32, tag="sc", bufs=2)
        wcol = half * A + p * P
        for e in range(EC):
            nc.tensor.matmul(
                ps[:, :512],
                lhsT=wqkv_sb[:, e, wcol : wcol + P],
                rhs=xt_sb[:, e, lt * 512 : (lt + 1) * 512],
                start=(e == 0),
                stop=(e == EC - 1),
            )
        Lsl = slice(lt * 512, (lt + 1) * 512)
        qs = work.tile([P, 512], BF16, tag="qs", bufs=3)
        nc.vector.tensor_copy(out=qs[:], in_=ps[:, :512])
        w = work.tile([P, 512], BF16, tag="w", bufs=3)
        t = work.tile([P, 512], BF16, tag="w", bufs=3)
        nc.vector.tensor_mul(w[:], qs[:], sin_sb[:, Lsl])
        wsw = work.tile([P, 512], BF16, tag="wsw", bufs=2)
        for blk in range(4):
            sb2 = blk ^ 1  # swap 32-row blocks pairwise
            nc.sync.dma_start(
                wsw[blk * 32 : (blk + 1) * 32, :], w[sb2 * 32 : (sb2 + 1) * 32, :]
            )
        nc.vector.tensor_mul(t[:], qs[:], cos_sb[:, Lsl])
        nc.vector.tensor_add(qk_sb[:, half, p, Lsl], t[:], wsw[:])

    # PE warmup: junk matmuls on a zeroed scratch tile keep the HAM
    # activity window busy while the first DMAs land, so the first real
    # matmuls run at full clock.
    wrm = work.tile([P, 128], BF16, tag="wrm", bufs=1)
    nc.vector.memset(wrm[:], 0.0)
    wps = psum.tile([65, 512], F32, tag="ot", bufs=4)
    for i in range(40):
        nc.tensor.matmul(
            wps[:, i % 4 * 128 : (i % 4 + 1) * 128],
            lhsT=wrm[:, 0:65],
            rhs=wrm[:],
            start=True,
            stop=True,
        )

    # prefix: just enough for attention(p0, lq0)'s first groups
    qk_half(0, 0, 1)
    qk_half(0, 0, 0)
    v_half(0)
    v_half(1)

    ycc = [
        dram.tile([512, E], F32, tag=f"ycc{t}", bufs=1, name=f"ycc{t}_{rep}")
        for t in range(LT)
    ]
    # collectives may not write IO tensors; ReduceScatter lands in a
    # Shared scratch tile, then a DMA moves it to y.
    yrs = [
        dram.tile([256, E], F32, tag=f"yrs{t}", bufs=1, name=f"yrs{t}_{rep}")
        for t in range(LT)
    ]

    # one oT2 staging tile ([128, pair, lq] feature-major) per lq tile,
    # filled incrementally by per-head gather DMAs after each normalize
    oT2s = {}

    def proj_chunk(lq, q4):
        def emit():
            oT2 = oT2s[lq]
            yps = psum.tile([P, 1024], F32, tag="sc", bufs=2)
            for c in range(A // P):
                for eh in range(2):
                    nc.tensor.matmul(
                        yps[:, eh * 512 : (eh + 1) * 512],
                        lhsT=oT2[:, c, q4 * P : (q4 + 1) * P],
                        rhs=wout_sb[:, c, eh * 512 : (eh + 1) * 512],
                        start=(c == 0),
                        stop=(c == A // P - 1),
                    )
            ysb = work.tile([P, E], F32, tag="ysb", bufs=2)
            nc.vector.tensor_copy(out=ysb[:], in_=yps[:])
            nc.sync.dma_start(ycc[lq][q4 * P : (q4 + 1) * P, :], ysb[:])
            if q4 == 3:
                if use_collective:
                    nc.gpsimd.collective_compute(
                        "ReduceScatter",
                        mybir.AluOpType.add,
                        replica_groups=[[2 * i, 2 * i + 1] for i in range(N_CORES // 2)],
                        ins=[ycc[lq][:].opt()],
                        outs=[yrs[lq][:].opt()],
                    )
                    nc.sync.dma_start(
                        y.ap()[lq * 256 : (lq + 1) * 256, :], yrs[lq][:]
                    )
                else:  # timing-analysis build: stand-in DMAs
                    nc.sync.dma_start(yrs[lq][:], ycc[lq][0:256, :])
                    nc.sync.dma_start(
                        y.ap()[lq * 256 : (lq + 1) * 256, :], yrs[lq][:]
                    )

        return emit

    def attention(p, lq, inject=None):
        inject = inject or {}
        Lq = slice(lq * 512, (lq + 1) * 512)
        ot0 = psum.tile([65, 512], F32, tag="ot", bufs=4)
        ot1 = psum.tile([65, 512], F32, tag="ot", bufs=4)
        ots = (ot0, ot1)
        groups = list(range(LKC))
        pss = {}

        def scores(gi):
            lk = gi
            ps = psum.tile([P, 1024], F32, tag="sc", bufs=2)
            pss[gi] = ps
            # pack_sim (timeline-analysis builds only): halve the moving
            # width so the cost model reflects the 2-head row-group
            # concurrency real hardware gets (auto tile_position 0/64).
            fw = 256 if pack_sim else 512
            for hh in range(2):
                nc.tensor.matmul(
                    ps[:, hh * 512 : hh * 512 + fw],
                    lhsT=qk_sb[hh * 64 : (hh + 1) * 64, 1, p, lk * P : (lk + 1) * P],
                    rhs=qk_sb[hh * 64 : (hh + 1) * 64, 0, p, Lq][:, :fw],
                    start=True,
                    stop=True,
                )

        def expav(gi):
            lk = gi
            ps = pss.pop(gi)
            at = work.tile([P, 1024], BF16, tag="at", bufs=4)
            nc.scalar.activation(at[:], ps[:], Exp, scale=scale)
            for hh in range(2):
                nc.tensor.matmul(
                    ots[hh][:],
                    lhsT=vaug_sb[:, lk, 2 * p + hh, :],
                    rhs=at[:, hh * 512 : (hh + 1) * 512],
                    start=(lk == 0),
                    stop=(lk == LKC - 1),
                )

        scores(0)
        for gi in range(LKC):
            if gi + 1 < LKC:
                scores(gi + 1)
            expav(gi)
            for fn in inject.get(gi, ()):
                fn()
        # softmax denominator + normalize: recip on PSUM row 64, move to
        # row 0, gpsimd broadcast + multiply (keeps the chain off DVE's
        # in-order queue), then per-head gather into the proj-ready oT2.
        for hh, otp in ((0, ot0), (1, ot1)):
            den = work.tile([65, 512], F32, tag="den", bufs=2)
            nc.vector.tensor_copy(out=den[64:65, :], in_=otp[64:65, :])
            rec = work.tile([65, 512], F32, tag="rec", bufs=2)
            nc.vector.reciprocal_approx_fast(rec[64:65, :], den[64:65, :])
            den0 = work.tile([1, 512], F32, tag="den0", bufs=2)
            nc.sync.dma_start(den0[0:1, :], rec[64:65, :])
            rbc = work.tile([64, 512], F32, tag="rbc", bufs=2)
            nc.gpsimd.partition_broadcast(rbc[:], den0[0:1, :])
            nc.vector.tensor_mul(ot_sb[:, 2 * p + hh, Lq], otp[0:64, :], rbc[:])
            nc.sync.dma_start(
                oT2s[lq][hh * 64 : (hh + 1) * 64, p, :], ot_sb[:, 2 * p + hh, Lq]
            )

    # ---- global piece schedule ----
    # unit (p, lq) runs LKC attention groups; pieces are injected after
    # specific groups to fill PE while ACT grinds through the exps.
    ngroups = LKC
    sched = {(p, lq): {} for p in range(NPAIR) for lq in range(LT)}

    def add(p, lq, gi, fn):
        sched[(p, lq)].setdefault(min(gi, ngroups - 1), []).append(fn)

    # remaining V halves: v(lt) must land before expav(gi=lt) of (p0,lq0)
    for lt in range(2, LKC):
        add(0, 0, max(0, lt - 2), lambda lt=lt: v_half(lt))
    # pair0 K halves: k(lt) feeds scores(gi=4*lt)
    for lt in range(1, LT):
        add(0, 0, max(0, 4 * lt - 5), lambda lt=lt: qk_half(0, lt, 1))
    # next pair's lq0-Q and all-K during unit (p, lq0)
    for p in range(NPAIR - 1):
        add(p, 0, 5, lambda p=p: qk_half(p + 1, 0, 0))
        for lt in range(LT):
            add(p, 0, 7 + 2 * lt, lambda p=p, lt=lt: qk_half(p + 1, lt, 1))
    # own pair's next-tile Q during unit (p, lq)
    for lq in range(LT - 1):
        for p in range(NPAIR):
            add(p, lq, 6 if lq == 0 else 3,
                lambda p=p, lq=lq: qk_half(p, lq + 1, 0))
    # proj chunk q4 of tile lq-1 late in unit (p=q4, lq): chunk 3 (with
    # the collective) latest, also keeping PE warm through the last
    # pair's normalize chain.
    for lq in range(1, LT):
        for q4 in range(4):
            add(q4, lq, 13 if q4 == 3 else 10, proj_chunk(lq - 1, q4))

    for lq in range(LT):
        oT2 = work.tile([P, A // P, 512], BF16, tag="oT2", bufs=2)
        oT2s[lq] = oT2
        for p in range(NPAIR):
            attention(p, lq, sched[(p, lq)])
    for q4 in range(4):
        proj_chunk(LT - 1, q4)()

    ctx.close()


@functools.lru_cache(maxsize=2)
def build_module(L=L_FULL, E=E_FULL, HC=H_FULL // 2, D=64, asserts=False,
                 debug_taps=False, use_collective=True, reps=1, version=1,
                 pack_sim=False):
    nc = bacc.Bacc(
        "TRN2",
        target_bir_lowering=False,
        debug=False,
        enable_asserts=asserts,
        num_devices=N_CORES,
    )
    A = HC * D
    xT = nc.dram_tensor("xT", [E, L], BF16, kind="ExternalInput")
    wqkv = nc.dram_tensor("wqkv", [E, 3 * A], BF16, kind="ExternalInput")
    wout_shape = [A, E] if version == 2 else [E, E]
    wout = nc.dram_tensor("wout", wout_shape, BF16, kind="ExternalInput")
    cosT = nc.dram_tensor("cosT", [128, L], BF16, kind="ExternalInput")
    sinT = nc.dram_tensor("sinT", [128, L], BF16, kind="ExternalInput")
    y_shape = [L // 2, E] if version == 2 else [L, E]
    y = nc.dram_tensor("y", y_shape, F32, kind="ExternalOutput")
    taps = None
    if debug_taps:
        assert version == 1
        taps = {
            "ot": nc.dram_tensor("ot_dbg", [64, HC, L], BF16, kind="ExternalOutput"),
            "ocat": nc.dram_tensor("ocat_dbg", [E, L], BF16, kind="ExternalOutput"),
            "qk": nc.dram_tensor("qk_dbg", [128, 2, HC // 2, L], BF16, kind="ExternalOutput"),
            "vaug": nc.dram_tensor(
                "vaug_dbg", [128, L // 128, HC, D + 1], BF16, kind="ExternalOutput"
            ),
        }
    with tile.TileContext(nc) as tc:
        for r in range(reps):
            if version == 2:
                _emit2(tc, nc, xT, wqkv, wout, cosT, sinT, y, L, E, HC, D,
                       use_collective=use_collective, rep=r, pack_sim=pack_sim)
            else:
                _emit(tc, nc, xT, wqkv, wout, cosT, sinT, y, L, E, HC, D,
                      taps=taps, use_collective=use_collective, rep=r)
    nc.compile()
    return nc


def _rope_tables(L, D, version=2):
    """cos/sin tables matching the de-interleaved weight layout.

    version 1 (32-granular): rows [0,32) = freqs 0-31 "x1" slots, rows
    [32,64) their "x2" partners; rotate-half = 32-row block swap.
    version 2 (16-granular): per 64-row head block, rows 0-15 = x1 of
    freqs 0-15, 16-31 = x2 of freqs 0-15, 32-47 = x1 of freqs 16-31,
    48-63 = x2 of freqs 16-31; rotate-half = 16-row swap within each
    32-row quadrant (DVE stream_shuffle).  sin is pre-signed (+ on x1
    slots, - on x2 slots).
    """
    half = D // 2
    inv_freq = 1.0 / (ROPE_THETA ** (np.arange(0, D, 2, dtype=np.float64) / D))
    freqs = np.arange(L, dtype=np.float64)[None, :] * inv_freq[:, None]  # [32, L]
    cos32 = np.cos(freqs)
    sin32 = np.sin(freqs)
    bf = ml_dtypes.bfloat16
    if version == 1:
        cos = np.tile(cos32, (128 // half, 1)).astype(bf)
        sin_block = np.concatenate([sin32, -sin32], axis=0)  # [64, L]
        sin = np.tile(sin_block, (2, 1)).astype(bf)
        return cos, sin
    cos64 = np.concatenate(
        [cos32[0:16], cos32[0:16], cos32[16:32], cos32[16:32]], axis=0
    )
    sin64 = np.concatenate(
        [sin32[0:16], -sin32[0:16], sin32[16:32], -sin32[16:32]], axis=0
    )
    cos = np.tile(cos64, (2, 1)).astype(bf)
    sin = np.tile(sin64, (2, 1)).astype(bf)
    return cos, sin


def _deint_cols(base, h, D, version=2):
    """Column indices of head h (offset base) in deinterleaved order."""
    cols = base + h * D + np.arange(D)
    if version == 1:
        return np.concatenate([cols[0::2], cols[1::2]])
    return np.concatenate(
        [cols[0:32:2], cols[1:32:2], cols[32:64:2], cols[33:64:2]]
    )


def make_core_inputs(x, w_qkv, w_out, H=H_FULL, D=64, version=1):
    """Per-core input dicts from the full (unsharded) fp32 inputs."""
    Bv, L, E = x.shape
    HC = H // (N_CORES // Bv)
    A_full = H * D
    bf = ml_dtypes.bfloat16
    cos, sin = _rope_tables(L, D, version=1)
    wout_bf = np.ascontiguousarray(w_out).astype(bf)
    in_maps = []
    for c in range(N_CORES):
        b, g = c // 2, c % 2
        if version == 2:  # own-half w_out rows (tensor-parallel split)
            wout_bf = np.ascontiguousarray(
                w_out[g * (HC * D) : (g + 1) * (HC * D), :]
            ).astype(bf)
        xT = np.ascontiguousarray(x[b].T).astype(bf)
        qcols = []
        kcols = []
        vcols = []
        for p in range(HC // 2):
            for hh in range(2):
                h = g * HC + 2 * p + hh
                qcols.append(_deint_cols(0, h, D, version=1))
                kcols.append(_deint_cols(A_full, h, D, version=1))
        for hl in range(HC):
            h = g * HC + hl
            vcols.append(2 * A_full + h * D + np.arange(D))
        cols = np.concatenate(qcols + kcols + vcols)
        wqkv_c = np.ascontiguousarray(w_qkv[:, cols]).astype(bf)
        in_maps.append(
            {
                "xT": xT,
                "wqkv": wqkv_c,
                "wout": wout_bf,
                "cosT": cos[:, :L].copy(),
                "sinT": sin[:, :L].copy(),
            }
        )
    return in_maps


def assemble_output(core_ys, Bv, L, E):
    """Reassemble full [B, L, E] from per-core outputs.

    v1: each pair's even core holds the full y.  v2 (ReduceScatter over
    the pair): even core holds rows [lq*512, lq*512+256) of each tile,
    odd core the next 256.
    """
    out = np.empty((Bv, L, E), dtype=np.float32)
    for b in range(Bv):
        ye = np.asarray(core_ys[2 * b])
        if ye.shape[0] == L:  # v1: full output on the even core
            out[b] = ye
            continue
        ye = ye.reshape(-1, 256, E)
        yo = np.asarray(core_ys[2 * b + 1]).reshape(-1, 256, E)
        out[b] = np.stack([ye, yo], axis=1).reshape(L, E)
    return out


def kernel(x, w_qkv, w_out):
    x = np.asarray(x)
    w_qkv = np.asarray(w_qkv)
    w_out = np.asarray(w_out)
    Bv, L, E = x.shape
    nc = build_module(L=L, E=E)
    in_maps = make_core_inputs(x, w_qkv, w_out)
    res = run_bass_kernel_spmd(nc, in_maps, core_ids=list(range(N_CORES)))
    return assemble_output([res.results[c]["y"] for c in range(N_CORES)], Bv, L, E)



# revision 2
# speedup vs baseline: 1.2598x; 1.2598x over previous
"""Multi-head self-attention with RoPE — Trainium2 Bass kernel, 8 NeuronCores.

Sharding: core c = 2*b + g handles batch b = c//2 and head-group g = c%2
(8 of the 16 heads).  No cross-core collectives: each core projects its
own half of the heads through the matching w_out row block into a
partial y [L, E] (fp32), and the host sums the two partials per batch.
Decoupling the cores keeps each NEFF's execution window free of
cross-core waits (collectives couple exec time to SPMD launch skew).

Per-core dataflow (matmuls bf16, fp32 PSUM accumulation):
  xT [E, L] bf16 (pre-transposed on host)
  QKV:   Q^T/K^T pair tiles via W-stationary matmuls; V natural [L, 512].
  RoPE:  weights pre-permuted on host to de-interleave even/odd dims, so
         rotate-half becomes a 32-partition block swap (SBUF->SBUF DMA);
         cos/sin multiplies + combine add on DVE.
  Attention (lq-outer, pair-inner):
    Scores: S^T half-tiles [Lk=128, Lq=512]; the two heads of a pair
         share a [128, 1024] PSUM tile (double buffered) per Lk chunk so
         exp runs as one wide ACT instruction overlapped with PE.
    Softmax: denominator via ones-column appended to V (PSUM partition 64
         of O^T); reciprocal_approx + gpsimd partition_broadcast.
    AV:  O^T[65, 512] += V_aug^T A^T over 16 Lk chunks; per-head O^T
         accumulators [65, 512] double buffered (psum: 4+2+2 = 8 banks).
    Normalized O^T lands in o2_sb [128, pair, L] (odd head moved to
         partitions 64-127 by a small SBUF->SBUF DMA) — proj-ready.
  Proj:  y[128-row chunk] = o2^T.T @ w_out_own [512, E], injected into
         the next lq tile's attention units so PE fills ACT-bound slack;
         partial y DMA'd out per chunk.
"""

import contextlib
import functools

import numpy as np
import ml_dtypes

import concourse.bass as bass
import concourse.mybir as mybir
import concourse.tile as tile
from concourse import bacc
from concourse.bass_utils import run_bass_kernel_spmd

BF16 = mybir.dt.bfloat16
F32 = mybir.dt.float32
N_CORES = 8
ROPE_THETA = 10000.0

B_FULL, L_FULL, E_FULL = 4, 2048, 1024
H_FULL = 16


def _emit3(tc, nc, xT, wqkv, wout, cosT, sinT, y, L, E, HC, D):
    P = 128
    EC = E // P                 # E chunks of 128 (contraction)
    NPAIR = HC // 2             # head pairs per core
    LT = L // 512               # 512-wide L tiles
    LKC = L // P                # 128-wide Lk chunks
    A = HC * D                  # local attention width (512)
    scale = 1.0 / float(np.sqrt(D))
    Exp = mybir.ActivationFunctionType.Exp

    ctx = contextlib.ExitStack()
    pool = ctx.enter_context(tc.tile_pool(name="sb", bufs=1))
    psum = ctx.enter_context(tc.tile_pool(name="ps", bufs=1, space="PSUM"))
    work = ctx.enter_context(tc.tile_pool(name="wk", bufs=1))

    # ---- persistent SBUF buffers ----
    xt_sb = pool.tile([P, EC, L], BF16, tag="xbuf")
    wqkv_sb = pool.tile([P, EC, 3 * A], BF16, tag="wqkv")
    wout_sb = pool.tile([P, A // P, E], BF16, tag="wout")   # own head rows
    cos_sb = pool.tile([P, L], BF16, tag="costab")
    sin_sb = pool.tile([P, L], BF16, tag="sintab")
    qk_sb = pool.tile([P, 2, NPAIR, L], BF16, tag="qk")      # [pair-rows, q/k, pair, L]
    vaug_sb = pool.tile([P, LKC, HC, D + 1], BF16, tag="vaug")
    o2_sb = pool.tile([P, NPAIR, L], BF16, tag="o2")         # normalized O^T, proj-ready

    nc.sync.dma_start(wqkv_sb[:], wqkv.ap().rearrange("(c p) n -> p c n", p=P))
    for xc in range(LT):  # x in L-chunks so the V matmuls start early
        nc.sync.dma_start(
            xt_sb[:, :, xc * 512 : (xc + 1) * 512],
            xT.ap()[:, xc * 512 : (xc + 1) * 512].rearrange("(c p) l -> p c l", p=P),
        )
    nc.sync.dma_start(cos_sb[:], cosT.ap())
    nc.sync.dma_start(sin_sb[:], sinT.ap())
    nc.sync.dma_start(wout_sb[:], wout.ap().rearrange("(c p) n -> p c n", p=P))

    # ones column for the softmax denominator
    nc.vector.memset(vaug_sb[:, :, :, D : D + 1], 1.0)

    # ---- V = x @ Wv, natural [L, A] layout, 2 L-chunks per PSUM tile ----
    for vg in range(LKC // 2):
        ps = psum.tile([P, 1024], F32, tag="sc", bufs=2)
        for i in range(2):
            lt = vg * 2 + i
            for e in range(EC):
                nc.tensor.matmul(
                    ps[:, i * 512 : (i + 1) * 512],
                    lhsT=xt_sb[:, e, lt * P : (lt + 1) * P],
                    rhs=wqkv_sb[:, e, 2 * A : 3 * A],
                    start=(e == 0),
                    stop=(e == EC - 1),
                )
        nc.vector.tensor_copy(
            out=vaug_sb[:, vg * 2 : (vg + 1) * 2, :, 0:D],
            in_=ps[:].rearrange("p (t h d) -> p t h d", h=HC, d=D),
        )

    # ---- Q^T / K^T + RoPE (multiplies on DVE; rotate-half = swap DMAs) ----
    for p in range(NPAIR):
        for lt in range(LT):
            ps = psum.tile([P, 1024], F32, tag="sc", bufs=2)
            for qk in range(2):
                wcol = qk * A + p * P
                for e in range(EC):
                    nc.tensor.matmul(
                        ps[:, qk * 512 : (qk + 1) * 512],
                        lhsT=wqkv_sb[:, e, wcol : wcol + P],
                        rhs=xt_sb[:, e, lt * 512 : (lt + 1) * 512],
                        start=(e == 0),
                        stop=(e == EC - 1),
                    )
            Lsl = slice(lt * 512, (lt + 1) * 512)
            tab = lambda sb: (
                sb[:, Lsl][:, None, :].to_broadcast([P, 2, 512])
            )
            qs = work.tile([P, 1024], BF16, tag="qs", bufs=3)
            nc.scalar.copy(out=qs[:], in_=ps[:])
            qs_v = qs[:].rearrange("p (q c) -> p q c", q=2)
            w = work.tile([P, 1024], BF16, tag="w", bufs=3)
            t = work.tile([P, 1024], BF16, tag="w", bufs=3)
            nc.vector.tensor_mul(w[:].rearrange("p (q c) -> p q c", q=2), qs_v, tab(sin_sb))
            nc.vector.tensor_mul(t[:].rearrange("p (q c) -> p q c", q=2), qs_v, tab(cos_sb))
            wsw = work.tile([P, 1024], BF16, tag="wsw", bufs=2)
            for blk in range(4):
                sb = blk ^ 1  # swap 32-row blocks pairwise
                nc.sync.dma_start(
                    wsw[blk * 32 : (blk + 1) * 32, :], w[sb * 32 : (sb + 1) * 32, :]
                )
            out_ap = qk_sb[:, :, p, Lsl]  # [P, 2, 512]
            nc.vector.tensor_add(
                out_ap,
                t[:].rearrange("p (q c) -> p q c", q=2),
                wsw[:].rearrange("p (q c) -> p q c", q=2),
            )

    # ---- output projection piece: y rows [lcol, lcol+128) ----
    def proj_piece(lq, q4):
        lcol = lq * 512 + q4 * P
        ps = psum.tile([P, 1024], F32, tag="sc", bufs=2)
        for eh in range(E // 512):
            for c in range(A // P):
                nc.tensor.matmul(
                    ps[:, eh * 512 : (eh + 1) * 512],
                    lhsT=o2_sb[:, c, lcol : lcol + P],
                    rhs=wout_sb[:, c, eh * 512 : (eh + 1) * 512],
                    start=(c == 0),
                    stop=(c == A // P - 1),
                )
        yt = work.tile([P, E], F32, tag="yt", bufs=2)
        nc.vector.tensor_copy(out=yt[:], in_=ps[:, :E])
        nc.sync.dma_start(y.ap()[lcol : lcol + P, :], yt[:])

    # ---- attention unit: pair p, 512-wide query tile lq ----
    def unit(p, lq, inject=None):
        inject = inject or {}
        Lq = slice(lq * 512, (lq + 1) * 512)
        otA = psum.tile([65, 512], F32, tag="otA", bufs=2)
        otB = psum.tile([65, 512], F32, tag="otB", bufs=2)
        ots = (otA, otB)
        for g in range(LKC):
            ps = psum.tile([P, 1024], F32, tag="sc", bufs=2)
            for hh in range(2):
                nc.tensor.matmul(
                    ps[:, hh * 512 : (hh + 1) * 512],
                    lhsT=qk_sb[hh * 64 : (hh + 1) * 64, 1, p, g * P : (g + 1) * P],
                    rhs=qk_sb[hh * 64 : (hh + 1) * 64, 0, p, Lq],
                    start=True,
                    stop=True,
                )
            at = work.tile([P, 1024], BF16, tag="at", bufs=4)
            nc.scalar.activation(at[:], ps[:], Exp, scale=scale)
            for hh in range(2):
                nc.tensor.matmul(
                    ots[hh][:],
                    lhsT=vaug_sb[:, g, 2 * p + hh, :],
                    rhs=at[:, hh * 512 : (hh + 1) * 512],
                    start=(g == 0),
                    stop=(g == LKC - 1),
                )
            for fn in inject.get(g, ()):
                fn()
        # softmax denominator: PSUM row 64 -> SBUF row 64 -> (DMA) row 0 ->
        # reciprocal -> broadcast to 64 partitions -> scale O^T.  Even head
        # lands in o2_sb[0:64] directly; odd head goes via a staging tile
        # and a partition-moving DMA into o2_sb[64:128].
        for hh, otp in ((0, otA), (1, otB)):
            den = work.tile([65, 512], F32, tag="den", bufs=2)
            nc.vector.tensor_copy(out=den[64:65, :], in_=otp[64:65, :])
            den0 = work.tile([1, 512], F32, tag="den0", bufs=2)
            nc.sync.dma_start(den0[0:1, :], den[64:65, :])
            rec0 = work.tile([1, 512], F32, tag="rec0", bufs=2)
            nc.vector.reciprocal_approx_fast(rec0[0:1, :], den0[0:1, :])
            rbc = work.tile([64, 512], F32, tag="rbc", bufs=2)
            nc.gpsimd.partition_broadcast(rbc[:], rec0[0:1, :])
            if hh == 0:
                nc.vector.tensor_mul(o2_sb[0:64, p, Lq], otp[0:64, :], rbc[:])
            else:
                tmp = work.tile([64, 512], BF16, tag="otmp", bufs=2)
                nc.vector.tensor_mul(tmp[:], otp[0:64, :], rbc[:])
                nc.sync.dma_start(o2_sb[64:128, p, Lq], tmp[:])

    # ---- schedule: lq-outer; proj pieces of tile lq-1 injected into the
    # units of tile lq (piece q4 into unit p=q4, after group 6) ----
    gi = min(6, LKC - 1)
    for lq in range(LT):
        for p in range(NPAIR):
            inj = {}
            if lq > 0:
                inj[gi] = [functools.partial(proj_piece, lq - 1, p)]
            unit(p, lq, inj)
    for q4 in range(4):
        proj_piece(LT - 1, q4)

    ctx.close()


@functools.lru_cache(maxsize=2)
def build_module(L=L_FULL, E=E_FULL, HC=H_FULL // 2, D=64, asserts=False):
    nc = bacc.Bacc(
        "TRN2",
        target_bir_lowering=False,
        debug=False,
        enable_asserts=asserts,
        num_devices=N_CORES,
    )
    A = HC * D
    xT = nc.dram_tensor("xT", [E, L], BF16, kind="ExternalInput")
    wqkv = nc.dram_tensor("wqkv", [E, 3 * A], BF16, kind="ExternalInput")
    wout = nc.dram_tensor("wout", [A, E], BF16, kind="ExternalInput")
    cosT = nc.dram_tensor("cosT", [128, L], BF16, kind="ExternalInput")
    sinT = nc.dram_tensor("sinT", [128, L], BF16, kind="ExternalInput")
    y = nc.dram_tensor("y", [L, E], F32, kind="ExternalOutput")
    with tile.TileContext(nc) as tc:
        _emit3(tc, nc, xT, wqkv, wout, cosT, sinT, y, L, E, HC, D)
    nc.compile()
    return nc


def _rope_tables(L, D):
    """cos/sin tables matching the de-interleaved weight layout.

    32-granular: rows [0,32) = freqs 0-31 "x1" slots, rows [32,64) their
    "x2" partners; rotate-half = 32-row block swap.  sin is pre-signed
    (+ on x1 slots, - on x2 slots).
    """
    half = D // 2
    inv_freq = 1.0 / (ROPE_THETA ** (np.arange(0, D, 2, dtype=np.float64) / D))
    freqs = np.arange(L, dtype=np.float64)[None, :] * inv_freq[:, None]  # [32, L]
    cos32 = np.cos(freqs)
    sin32 = np.sin(freqs)
    bf = ml_dtypes.bfloat16
    cos = np.tile(cos32, (128 // half, 1)).astype(bf)
    sin_block = np.concatenate([sin32, -sin32], axis=0)  # [64, L]
    sin = np.tile(sin_block, (2, 1)).astype(bf)
    return cos, sin


def _deint_cols(base, h, D):
    """Column indices of head h (offset base) in deinterleaved order."""
    cols = base + h * D + np.arange(D)
    return np.concatenate([cols[0::2], cols[1::2]])


def make_core_inputs(x, w_qkv, w_out, H=H_FULL, D=64):
    """Per-core input dicts from the full (unsharded) fp32 inputs."""
    Bv, L, E = x.shape
    HC = H // (N_CORES // Bv)
    A_full = H * D
    bf = ml_dtypes.bfloat16
    cos, sin = _rope_tables(L, D)
    in_maps = []
    for c in range(N_CORES):
        b, g = c // 2, c % 2
        # own-half w_out rows (tensor-parallel split over heads)
        wout_bf = np.ascontiguousarray(
            w_out[g * (HC * D) : (g + 1) * (HC * D), :]
        ).astype(bf)
        xT = np.ascontiguousarray(x[b].T).astype(bf)
        qcols = []
        kcols = []
        vcols = []
        for p in range(HC // 2):
            for hh in range(2):
                h = g * HC + 2 * p + hh
                qcols.append(_deint_cols(0, h, D))
                kcols.append(_deint_cols(A_full, h, D))
        for hl in range(HC):
            h = g * HC + hl
            vcols.append(2 * A_full + h * D + np.arange(D))
        cols = np.concatenate(qcols + kcols + vcols)
        wqkv_c = np.ascontiguousarray(w_qkv[:, cols]).astype(bf)
        in_maps.append(
            {
                "xT": xT,
                "wqkv": wqkv_c,
                "wout": wout_bf,
                "cosT": cos[:, :L].copy(),
                "sinT": sin[:, :L].copy(),
            }
        )
    return in_maps


def assemble_output(core_ys, Bv, L, E):
    """Full [B, L, E] from per-core partial y: sum each batch pair."""
    out = np.empty((Bv, L, E), dtype=np.float32)
    for b in range(Bv):
        out[b] = np.asarray(core_ys[2 * b]) + np.asarray(core_ys[2 * b + 1])
    return out


def kernel(x, w_qkv, w_out):
    x = np.asarray(x)
    w_qkv = np.asarray(w_qkv)
    w_out = np.asarray(w_out)
    Bv, L, E = x.shape
    nc = build_module(L=L, E=E)
    in_maps = make_core_inputs(x, w_qkv, w_out)
    res = run_bass_kernel_spmd(nc, in_maps, core_ids=list(range(N_CORES)))
    return assemble_output([res.results[c]["y"] for c in range(N_CORES)], Bv, L, E)


# revision 4
# speedup vs baseline: 1.2644x; 1.0037x over previous
"""Multi-head self-attention with RoPE — Trainium2 Bass kernel, 8 NeuronCores.

Sharding: core c = 2*b + g handles batch b = c//2 and head-group g = c%2
(8 of the 16 heads).  No cross-core collectives: each core projects its
own half of the heads through the matching w_out row block into a
partial y [L, E] (fp32), and the host sums the two partials per batch.
Decoupling the cores keeps each NEFF's execution window free of
cross-core waits (collectives couple exec time to SPMD launch skew).

Per-core dataflow (matmuls bf16, fp32 PSUM accumulation):
  xT [E, L] bf16 (pre-transposed on host)
  QKV:   Q^T/K^T pair tiles via W-stationary matmuls; V natural [L, 512].
  RoPE:  weights pre-permuted on host to de-interleave even/odd dims, so
         rotate-half becomes a 32-partition block swap (SBUF->SBUF DMA);
         cos/sin multiplies + combine add on DVE.
  Attention (lq-outer, pair-inner):
    Scores: S^T half-tiles [Lk=128, Lq=512]; the two heads of a pair
         share a [128, 1024] PSUM tile (double buffered) per Lk chunk so
         exp runs as one wide ACT instruction overlapped with PE.
    Softmax: denominator via ones-column appended to V (PSUM partition 64
         of O^T); reciprocal_approx + gpsimd partition_broadcast.
    AV:  O^T[65, 512] += V_aug^T A^T over 16 Lk chunks; per-head O^T
         accumulators [65, 512] double buffered (psum: 4+2+2 = 8 banks).
    Normalized O^T lands in o2_sb [128, pair, L] (odd head moved to
         partitions 64-127 by a small SBUF->SBUF DMA) — proj-ready.
  Proj:  y[128-row chunk] = o2^T.T @ w_out_own [512, E], injected into
         the next lq tile's attention units so PE fills ACT-bound slack;
         partial y DMA'd out per chunk.
"""

import contextlib
import functools

import numpy as np
import ml_dtypes

import concourse.bass as bass
import concourse.mybir as mybir
import concourse.tile as tile
from concourse import bacc
from concourse.bass_utils import run_bass_kernel_spmd

BF16 = mybir.dt.bfloat16
F32 = mybir.dt.float32
N_CORES = 8
ROPE_THETA = 10000.0

B_FULL, L_FULL, E_FULL = 4, 2048, 1024
H_FULL = 16


def _emit3(tc, nc, xT, wqkv, wout, cosT, sinT, y, L, E, HC, D):
    P = 128
    EC = E // P                 # E chunks of 128 (contraction)
    NPAIR = HC // 2             # head pairs per core
    LT = L // 512               # 512-wide L tiles
    LKC = L // P                # 128-wide Lk chunks
    A = HC * D                  # local attention width (512)
    scale = 1.0 / float(np.sqrt(D))
    Exp = mybir.ActivationFunctionType.Exp

    ctx = contextlib.ExitStack()
    pool = ctx.enter_context(tc.tile_pool(name="sb", bufs=1))
    psum = ctx.enter_context(tc.tile_pool(name="ps", bufs=1, space="PSUM"))
    work = ctx.enter_context(tc.tile_pool(name="wk", bufs=1))

    # ---- persistent SBUF buffers ----
    xt_sb = pool.tile([P, EC, L], BF16, tag="xbuf")
    wqkv_sb = pool.tile([P, EC, 3 * A], BF16, tag="wqkv")
    wout_sb = pool.tile([P, A // P, E], BF16, tag="wout")   # own head rows
    cos_sb = pool.tile([P, L], BF16, tag="costab")
    sin_sb = pool.tile([P, L], BF16, tag="sintab")
    qk_sb = pool.tile([P, 2, NPAIR, L], BF16, tag="qk")      # [pair-rows, q/k, pair, L]
    vaug_sb = pool.tile([P, LKC, HC, D + 1], BF16, tag="vaug")
    o2_sb = pool.tile([P, NPAIR, L], BF16, tag="o2")         # normalized O^T, proj-ready

    # Input DMAs spread across both HWDGE rings (sync=SP, scalar=ACT) and
    # SWDGE (gpsimd) so the first V matmul only waits for the V-column
    # slice of w_qkv plus the first x chunk (~4µs), not the full 8.4MB.
    wq = wqkv.ap().rearrange("(c p) n -> p c n", p=P)
    nc.sync.dma_start(wqkv_sb[:, :, 2 * A : 3 * A], wq[:, :, 2 * A : 3 * A])
    for xc in range(LT):  # x in L-chunks so the V matmuls start early
        nc.scalar.dma_start(
            xt_sb[:, :, xc * 512 : (xc + 1) * 512],
            xT.ap()[:, xc * 512 : (xc + 1) * 512].rearrange("(c p) l -> p c l", p=P),
        )
    nc.sync.dma_start(wqkv_sb[:, :, 0 : 2 * A], wq[:, :, 0 : 2 * A])
    nc.gpsimd.dma_start(cos_sb[:], cosT.ap())
    nc.gpsimd.dma_start(sin_sb[:], sinT.ap())
    nc.gpsimd.dma_start(wout_sb[:], wout.ap().rearrange("(c p) n -> p c n", p=P))

    # ones column for the softmax denominator
    nc.vector.memset(vaug_sb[:, :, :, D : D + 1], 1.0)

    # ---- V = x @ Wv, natural [L, A] layout, 2 L-chunks per PSUM tile ----
    for vg in range(LKC // 2):
        ps = psum.tile([P, 1024], F32, tag="sc", bufs=2)
        for i in range(2):
            lt = vg * 2 + i
            for e in range(EC):
                nc.tensor.matmul(
                    ps[:, i * 512 : (i + 1) * 512],
                    lhsT=xt_sb[:, e, lt * P : (lt + 1) * P],
                    rhs=wqkv_sb[:, e, 2 * A : 3 * A],
                    start=(e == 0),
                    stop=(e == EC - 1),
                )
        nc.vector.tensor_copy(
            out=vaug_sb[:, vg * 2 : (vg + 1) * 2, :, 0:D],
            in_=ps[:].rearrange("p (t h d) -> p t h d", h=HC, d=D),
        )

    # ---- Q^T / K^T + RoPE (multiplies on DVE; rotate-half = swap DMAs) ----
    for p in range(NPAIR):
        for lt in range(LT):
            ps = psum.tile([P, 1024], F32, tag="sc", bufs=2)
            for qk in range(2):
                wcol = qk * A + p * P
                for e in range(EC):
                    nc.tensor.matmul(
                        ps[:, qk * 512 : (qk + 1) * 512],
                        lhsT=wqkv_sb[:, e, wcol : wcol + P],
                        rhs=xt_sb[:, e, lt * 512 : (lt + 1) * 512],
                        start=(e == 0),
                        stop=(e == EC - 1),
                    )
            Lsl = slice(lt * 512, (lt + 1) * 512)
            tab = lambda sb: (
                sb[:, Lsl][:, None, :].to_broadcast([P, 2, 512])
            )
            qs = work.tile([P, 1024], BF16, tag="qs", bufs=3)
            nc.scalar.copy(out=qs[:], in_=ps[:])
            qs_v = qs[:].rearrange("p (q c) -> p q c", q=2)
            w = work.tile([P, 1024], BF16, tag="w", bufs=3)
            t = work.tile([P, 1024], BF16, tag="w", bufs=3)
            nc.vector.tensor_mul(w[:].rearrange("p (q c) -> p q c", q=2), qs_v, tab(sin_sb))
            nc.vector.tensor_mul(t[:].rearrange("p (q c) -> p q c", q=2), qs_v, tab(cos_sb))
            wsw = work.tile([P, 1024], BF16, tag="wsw", bufs=2)
            for blk in range(4):
                sb = blk ^ 1  # swap 32-row blocks pairwise
                nc.sync.dma_start(
                    wsw[blk * 32 : (blk + 1) * 32, :], w[sb * 32 : (sb + 1) * 32, :]
                )
            out_ap = qk_sb[:, :, p, Lsl]  # [P, 2, 512]
            nc.vector.tensor_add(
                out_ap,
                t[:].rearrange("p (q c) -> p q c", q=2),
                wsw[:].rearrange("p (q c) -> p q c", q=2),
            )

    # ---- output projection piece: y rows [lcol, lcol+128) ----
    def proj_piece(lq, q4):
        lcol = lq * 512 + q4 * P
        ps = psum.tile([P, 1024], F32, tag="sc", bufs=2)
        for eh in range(E // 512):
            for c in range(A // P):
                nc.tensor.matmul(
                    ps[:, eh * 512 : (eh + 1) * 512],
                    lhsT=o2_sb[:, c, lcol : lcol + P],
                    rhs=wout_sb[:, c, eh * 512 : (eh + 1) * 512],
                    start=(c == 0),
                    stop=(c == A // P - 1),
                )
        yt = work.tile([P, E], F32, tag="yt", bufs=2)
        nc.vector.tensor_copy(out=yt[:], in_=ps[:, :E])
        nc.sync.dma_start(y.ap()[lcol : lcol + P, :], yt[:])

    # ---- attention unit: pair p, 512-wide query tile lq ----
    def unit(p, lq, inject=None):
        inject = inject or {}
        Lq = slice(lq * 512, (lq + 1) * 512)
        otA = psum.tile([65, 512], F32, tag="otA", bufs=2)
        otB = psum.tile([65, 512], F32, tag="otB", bufs=2)
        ots = (otA, otB)
        pss = {}

        def scores(g):
            ps = psum.tile([P, 1024], F32, tag="sc", bufs=2)
            pss[g] = ps
            for hh in range(2):
                nc.tensor.matmul(
                    ps[:, hh * 512 : (hh + 1) * 512],
                    lhsT=qk_sb[hh * 64 : (hh + 1) * 64, 1, p, g * P : (g + 1) * P],
                    rhs=qk_sb[hh * 64 : (hh + 1) * 64, 0, p, Lq],
                    start=True,
                    stop=True,
                )

        # scores staggered one group ahead of AV so PE never queues
        # behind the exp it feeds.
        scores(0)
        for g in range(LKC):
            if g + 1 < LKC:
                scores(g + 1)
            ps = pss.pop(g)
            at = work.tile([P, 1024], BF16, tag="at", bufs=4)
            nc.scalar.activation(at[:], ps[:], Exp, scale=scale)
            for hh in range(2):
                nc.tensor.matmul(
                    ots[hh][:],
                    lhsT=vaug_sb[:, g, 2 * p + hh, :],
                    rhs=at[:, hh * 512 : (hh + 1) * 512],
                    start=(g == 0),
                    stop=(g == LKC - 1),
                )
            for fn in inject.get(g, ()):
                fn()
        # softmax denominator: PSUM row 64 -> SBUF row 64 -> (DMA) row 0 ->
        # reciprocal -> broadcast to 64 partitions -> scale O^T.  Even head
        # lands in o2_sb[0:64] directly; odd head goes via a staging tile
        # and a partition-moving DMA into o2_sb[64:128].
        for hh, otp in ((0, otA), (1, otB)):
            den = work.tile([65, 512], F32, tag="den", bufs=2)
            nc.vector.tensor_copy(out=den[64:65, :], in_=otp[64:65, :])
            den0 = work.tile([1, 512], F32, tag="den0", bufs=2)
            nc.sync.dma_start(den0[0:1, :], den[64:65, :])
            rec0 = work.tile([1, 512], F32, tag="rec0", bufs=2)
            nc.vector.reciprocal_approx_fast(rec0[0:1, :], den0[0:1, :])
            rbc = work.tile([64, 512], F32, tag="rbc", bufs=2)
            nc.gpsimd.partition_broadcast(rbc[:], rec0[0:1, :])
            if hh == 0:
                nc.vector.tensor_mul(o2_sb[0:64, p, Lq], otp[0:64, :], rbc[:])
            else:
                tmp = work.tile([64, 512], BF16, tag="otmp", bufs=2)
                nc.vector.tensor_mul(tmp[:], otp[0:64, :], rbc[:])
                nc.sync.dma_start(o2_sb[64:128, p, Lq], tmp[:])

    # ---- schedule: lq-outer; proj pieces of tile lq-1 injected into the
    # units of tile lq (piece q4 into unit p=q4, after group 6) ----
    gi = min(6, LKC - 1)
    for lq in range(LT):
        for p in range(NPAIR):
            inj = {}
            if lq > 0:
                inj[gi] = [functools.partial(proj_piece, lq - 1, p)]
            unit(p, lq, inj)
    for q4 in range(4):
        proj_piece(LT - 1, q4)

    ctx.close()


@functools.lru_cache(maxsize=2)
def build_module(L=L_FULL, E=E_FULL, HC=H_FULL // 2, D=64, asserts=False):
    nc = bacc.Bacc(
        "TRN2",
        target_bir_lowering=False,
        debug=False,
        enable_asserts=asserts,
        num_devices=N_CORES,
    )
    A = HC * D
    xT = nc.dram_tensor("xT", [E, L], BF16, kind="ExternalInput")
    wqkv = nc.dram_tensor("wqkv", [E, 3 * A], BF16, kind="ExternalInput")
    wout = nc.dram_tensor("wout", [A, E], BF16, kind="ExternalInput")
    cosT = nc.dram_tensor("cosT", [128, L], BF16, kind="ExternalInput")
    sinT = nc.dram_tensor("sinT", [128, L], BF16, kind="ExternalInput")
    y = nc.dram_tensor("y", [L, E], F32, kind="ExternalOutput")
    with tile.TileContext(nc) as tc:
        _emit3(tc, nc, xT, wqkv, wout, cosT, sinT, y, L, E, HC, D)
    nc.compile()
    return nc


def _rope_tables(L, D):
    """cos/sin tables matching the de-interleaved weight layout.

    32-granular: rows [0,32) = freqs 0-31 "x1" slots, rows [32,64) their
    "x2" partners; rotate-half = 32-row block swap.  sin is pre-signed
    (+ on x1 slots, - on x2 slots).
    """
    half = D // 2
    inv_freq = 1.0 / (ROPE_THETA ** (np.arange(0, D, 2, dtype=np.float64) / D))
    freqs = np.arange(L, dtype=np.float64)[None, :] * inv_freq[:, None]  # [32, L]
    cos32 = np.cos(freqs)
    sin32 = np.sin(freqs)
    bf = ml_dtypes.bfloat16
    cos = np.tile(cos32, (128 // half, 1)).astype(bf)
    sin_block = np.concatenate([sin32, -sin32], axis=0)  # [64, L]
    sin = np.tile(sin_block, (2, 1)).astype(bf)
    return cos, sin


def _deint_cols(base, h, D):
    """Column indices of head h (offset base) in deinterleaved order."""
    cols = base + h * D + np.arange(D)
    return np.concatenate([cols[0::2], cols[1::2]])


def make_core_inputs(x, w_qkv, w_out, H=H_FULL, D=64):
    """Per-core input dicts from the full (unsharded) fp32 inputs."""
    Bv, L, E = x.shape
    HC = H // (N_CORES // Bv)
    A_full = H * D
    bf = ml_dtypes.bfloat16
    cos, sin = _rope_tables(L, D)
    in_maps = []
    for c in range(N_CORES):
        b, g = c // 2, c % 2
        # own-half w_out rows (tensor-parallel split over heads)
        wout_bf = np.ascontiguousarray(
            w_out[g * (HC * D) : (g + 1) * (HC * D), :]
        ).astype(bf)
        xT = np.ascontiguousarray(x[b].T).astype(bf)
        qcols = []
        kcols = []
        vcols = []
        for p in range(HC // 2):
            for hh in range(2):
                h = g * HC + 2 * p + hh
                qcols.append(_deint_cols(0, h, D))
                kcols.append(_deint_cols(A_full, h, D))
        for hl in range(HC):
            h = g * HC + hl
            vcols.append(2 * A_full + h * D + np.arange(D))
        cols = np.concatenate(qcols + kcols + vcols)
        wqkv_c = np.ascontiguousarray(w_qkv[:, cols]).astype(bf)
        in_maps.append(
            {
                "xT": xT,
                "wqkv": wqkv_c,
                "wout": wout_bf,
                "cosT": cos[:, :L].copy(),
                "sinT": sin[:, :L].copy(),
            }
        )
    return in_maps


def assemble_output(core_ys, Bv, L, E):
    """Full [B, L, E] from per-core partial y: sum each batch pair."""
    out = np.empty((Bv, L, E), dtype=np.float32)
    for b in range(Bv):
        out[b] = np.asarray(core_ys[2 * b]) + np.asarray(core_ys[2 * b + 1])
    return out


def kernel(x, w_qkv, w_out):
    x = np.asarray(x)
    w_qkv = np.asarray(w_qkv)
    w_out = np.asarray(w_out)
    Bv, L, E = x.shape
    nc = build_module(L=L, E=E)
    in_maps = make_core_inputs(x, w_qkv, w_out)
    res = run_bass_kernel_spmd(nc, in_maps, core_ids=list(range(N_CORES)))
    return assemble_output([res.results[c]["y"] for c in range(N_CORES)], Bv, L, E)
